# revision 12
# baseline (speedup 1.0000x reference)
"""Trainium2 Bass kernel for nn_MultiHeadAttention (B=4, N=2048, C=512, H=8).

Returns (out [B,N,C] f32, attn_mean [B,N,N] f32) like the reference.

Sharding: 8 cores = 4 batches x 2 query-halves. Each core computes all 8
heads for its (batch, 1024-query-row) slice; no collectives. The query-half
selection is done by rolling x/mask on the host so the SPMD program is
identical on every core (each core handles rows [0:NQ) of its rolled input;
attn_mean columns are rolled back on the host).

Per-core dataflow (matmuls bf16, PSUM accumulation f32):
  setup: x^T via PE transpose; Q^T (x0.125, +bias), K^T (+bias) in [d, n]
         layout; V masked by the key mask with an extra mask column (so the
         P@V matmul also yields the masked softmax denominator Z).
  per head:  S^T[k,q] matmul -> exp -> P~^T (unnormalized, bf16)
             P~^T @ [V|m] -> O_unnorm and Z; rz = 1/Z; O = O_unnorm * rz
             S[q,k] matmul -> exp -> P~; A[q,k] += P~ * rz   (fused DVE op)
  tail:  A *= mask (zero masked columns), DMA out (bf16; host /8 -> mean)
         out = (O @ W_proj + b_proj) via PE transposes of O.
"""

import numpy as np
import ml_dtypes

B, NFULL, C = 4, 2048, 512
H, D = 8, 64
NQ = NFULL // 2  # query rows per core
N_CORES = 8
SCALE = D ** -0.5

_prog_cache = {}


def _pin_act_tables():
    import concourse.bacc as bacc_mod
    import concourse.mybir as mybir
    if getattr(bacc_mod, "_act_tables_pinned", False):
        return
    orig = bacc_mod.get_activation_tables

    def pinned(arch):
        t = dict(orig(arch))
        keep = "natural_log_exp_and_others"
        drop = {mybir.ActivationFunctionType.Exp,
                mybir.ActivationFunctionType.Ln}
        if keep in t:
            for name in t:
                if name != keep:
                    t[name] = t[name] - drop
        return t

    bacc_mod.get_activation_tables = pinned
    bacc_mod._act_tables_pinned = True


def _build_program(nfull=NFULL, nq=NQ):
    import concourse.bass as bass
    import concourse.tile as tile
    import concourse.mybir as mybir
    from concourse import bacc
    from concourse.masks import make_identity
    _pin_act_tables()

    dt = mybir.dt
    FP32 = dt.float32
    BF16 = dt.bfloat16
    AF = mybir.ActivationFunctionType
    OP = mybir.AluOpType

    NT = nfull // 128      # k/v row tiles
    CT = C // 128          # channel tiles (4)
    QT = nq // 128         # query tiles per core
    HP = H // 2            # head pairs (4)
    KCW = min(1024, nfull)       # k chunk width for the [q,k] side
    KC = nfull // KCW            # chunks
    W5 = min(512, nq)            # matmul free width for [d,n] outputs
    NQ5 = nq // W5
    S5 = min(512, KCW)           # matmul free width for S[q,k]
    NS5 = KCW // S5

    nc = bacc.Bacc("TRN2", target_bir_lowering=False, debug=False,
                   num_devices=N_CORES)

    xb = nc.dram_tensor("xb", [nfull, C], FP32, kind="ExternalInput").ap()
    maskv = nc.dram_tensor("maskv", [nfull], dt.int32, kind="ExternalInput").ap()
    wqkv = nc.dram_tensor("wqkv", [C, 3 * C], FP32, kind="ExternalInput").ap()
    bqkv = nc.dram_tensor("bqkv", [3 * C], FP32, kind="ExternalInput").ap()
    wproj = nc.dram_tensor("wproj", [C, C], FP32, kind="ExternalInput").ap()
    bproj = nc.dram_tensor("bproj", [C], FP32, kind="ExternalInput").ap()
    out_o = nc.dram_tensor("out_o", [nq, C], FP32, kind="ExternalOutput").ap()
    out_a = nc.dram_tensor("out_a", [nq, nfull], BF16, kind="ExternalOutput").ap()

    with tile.TileContext(nc) as tc:
        from contextlib import ExitStack
        ctx = ExitStack()
        with ctx:
            # ---------- persistent pools (each tag: its own bufs) ----------
            pers = ctx.enter_context(tc.tile_pool(name="pers", bufs=1))

            # PSUM pools
            sp = ctx.enter_context(tc.tile_pool(name="sp", bufs=3, space="PSUM"))
            pvp = ctx.enter_context(tc.tile_pool(name="pvp", bufs=1, space="PSUM"))
            ovp = ctx.enter_context(tc.tile_pool(name="ovp", bufs=1, space="PSUM"))

            id32 = pers.tile([128, 128], FP32, tag="id32")
            make_identity(nc, id32[:])
            id16 = pers.tile([128, 128], BF16, tag="id16")
            make_identity(nc, id16[:])

            mfT = pers.tile([128, NT], FP32, tag="mfT")     # mask [k%128, kt]
            maskbc = pers.tile([128, nfull], BF16, tag="maskbc")
            ones8 = pers.tile([128, H], BF16, tag="ones8")
            bpb = pers.tile([128, C], FP32, tag="bpb")      # bproj broadcast
            onesrow = pers.tile([1, nfull], BF16, tag="onesrow")
            nc.vector.memset(ones8[:], 1.0)
            nc.vector.memset(onesrow[:], 1.0)

            QTt = [pers.tile([128, nq], BF16, tag="qt", bufs=HP, name=f"qt{i}")
                   for i in range(HP)]
            KTt = [pers.tile([128, nfull], BF16, tag="kt", bufs=HP, name=f"kt{i}")
                   for i in range(HP)]
            Vt = pers.tile([128, NT, H, D + 1], BF16, tag="vt")
            WPt = [pers.tile([128, C], BF16, tag="wp", bufs=CT, name=f"wp{i}")
                   for i in range(CT)]

            # ---------- setup (transient pools) ----------
            with tc.tile_pool(name="stage", bufs=2) as stg, \
                 tc.tile_pool(name="sxt", bufs=1) as sxt, \
                 tc.tile_pool(name="svec", bufs=1) as svec:

                # mask: strided [128, NT] + row + broadcast
                mi = svec.tile([128, NT], dt.int32, tag="mi")
                nc.sync.dma_start(mi[:], maskv.rearrange("(t p) -> p t", p=128))
                nc.vector.tensor_copy(mfT[:], mi[:])
                mri = svec.tile([1, nfull], dt.int32, tag="mri")
                nc.sync.dma_start(mri[:], maskv.unsqueeze(0))
                mrf = svec.tile([1, nfull], FP32, tag="mrf")
                nc.vector.tensor_copy(mrf[:], mri[:])
                mrow16 = svec.tile([1, nfull], BF16, tag="mrow16")
                nc.vector.tensor_copy(mrow16[:], mrf[:])
                # broadcast along partitions via ones-column x row matmuls
                bw = min(512, nfull)
                for kb in range(nfull // bw):
                    bb = pvp.tile([128, bw], FP32, tag="pv", name="bb")
                    nc.tensor.matmul(bb[:], onesrow[0:1, 0:128],
                                     mrow16[0:1, kb * bw:(kb + 1) * bw])
                    nc.vector.tensor_copy(maskbc[:, kb * bw:(kb + 1) * bw],
                                          bb[:])
                bprow = svec.tile([1, C], FP32, tag="bprow")
                nc.sync.dma_start(bprow[:], bproj.unsqueeze(0))
                ones32 = svec.tile([1, 128], FP32, tag="ones32")
                nc.vector.memset(ones32[:], 1.0)
                bb2 = pvp.tile([128, C], FP32, tag="pv", name="bb2")
                nc.tensor.matmul(bb2[:], ones32[:], bprow[:])
                nc.vector.tensor_copy(bpb[:], bb2[:])

                # qkv bias vectors [128, 12] (column t = b[128t:128(t+1)])
                bq12 = svec.tile([128, 3 * CT], FP32, tag="bq12")
                nc.sync.dma_start(bq12[:], bqkv.rearrange("(t p) -> p t", p=128))
                bqs = svec.tile([128, CT], FP32, tag="bqs")  # 0.125 * b_q
                nc.vector.tensor_scalar(bqs[:], bq12[:, 0:CT], SCALE, None,
                                        op0=OP.mult)

                # x -> SBUF, transpose -> xT (bf16)
                xT = [sxt.tile([128, nfull], BF16, tag="xT", bufs=CT, name=f"xT{i}")
                      for i in range(CT)]
                for nt in range(NT):
                    xs = stg.tile([128, C], FP32, tag="xs")
                    nc.sync.dma_start(xs[:], xb[nt * 128:(nt + 1) * 128, :])
                    for ct in range(CT):
                        tp = pvp.tile([128, 128], FP32, tag="pv")
                        nc.tensor.transpose(tp[:],
                                            xs[:, ct * 128:(ct + 1) * 128],
                                            id32[:])
                        nc.vector.tensor_copy(
                            xT[ct][:, nt * 128:(nt + 1) * 128], tp[:])

                # W_qkv -> bf16 (Q columns pre-scaled by 0.125)
                Wc = [sxt.tile([128, 3 * C], BF16, tag="wc", bufs=CT, name=f"wc{i}")
                      for i in range(CT)]
                bvrow = sxt.tile([1, C], BF16, tag="bvrow")
                bvf = svec.tile([1, 3 * C], FP32, tag="bvf")
                nc.sync.dma_start(bvf[:], bqkv.unsqueeze(0))
                nc.vector.tensor_copy(bvrow[:], bvf[0:1, 2 * C:3 * C])
                for ct in range(CT):
                    ws = stg.tile([128, 3 * C], FP32, tag="ws")
                    nc.sync.dma_start(ws[:], wqkv[ct * 128:(ct + 1) * 128, :])
                    nc.vector.tensor_scalar(Wc[ct][:, 0:C], ws[:, 0:C], SCALE,
                                            None, op0=OP.mult)
                    nc.vector.tensor_copy(Wc[ct][:, C:3 * C], ws[:, C:3 * C])

                # W_proj -> bf16
                for ct in range(CT):
                    wps = stg.tile([128, C], FP32, tag="wps")
                    nc.sync.dma_start(wps[:], wproj[ct * 128:(ct + 1) * 128, :])
                    nc.vector.tensor_copy(WPt[ct][:], wps[:])

                # Q^T tiles [d-pair 128, nq]: 0.125*(Wq^T x^T) + 0.125*bq
                # (0.125 pre-folded into Wc's Q columns and bqs)
                for j in range(HP):
                    ps = sp.tile([128, nq], FP32, tag="s")
                    for ct in range(CT):
                        for q5 in range(NQ5):
                            nc.tensor.matmul(
                                ps[:, q5 * W5:(q5 + 1) * W5],
                                Wc[ct][:, j * 128:(j + 1) * 128],
                                xT[ct][:, q5 * W5:(q5 + 1) * W5],
                                start=(ct == 0), stop=(ct == CT - 1))
                    nc.vector.tensor_scalar(QTt[j][:], ps[:],
                                            bqs[:, j:j + 1], None, op0=OP.add)

                # K^T tiles [d-pair 128, nfull]
                for j in range(HP):
                    for kc in range(nfull // nq):
                        ps = sp.tile([128, nq], FP32, tag="s")
                        for ct in range(CT):
                            for q5 in range(NQ5):
                                o = kc * nq + q5 * W5
                                nc.tensor.matmul(
                                    ps[:, q5 * W5:(q5 + 1) * W5],
                                    Wc[ct][:, C + j * 128:C + (j + 1) * 128],
                                    xT[ct][:, o:o + W5],
                                    start=(ct == 0), stop=(ct == CT - 1))
                        nc.vector.tensor_scalar(
                            KTt[j][:, kc * nq:(kc + 1) * nq], ps[:],
                            bq12[:, CT + j:CT + j + 1], None, op0=OP.add)

                # V tiles: masked, with mask column at d=64
                for nt in range(NT):
                    ps = pvp.tile([128, C], FP32, tag="pv")
                    for ct in range(CT):
                        nc.tensor.matmul(ps[:],
                                         xT[ct][:, nt * 128:(nt + 1) * 128],
                                         Wc[ct][:, 2 * C:3 * C],
                                         start=(ct == 0), stop=False)
                    nc.tensor.matmul(ps[:],
                                     onesrow[0:1, nt * 128:(nt + 1) * 128],
                                     bvrow[0:1, :], start=False, stop=True)
                    nc.vector.tensor_scalar(
                        Vt[:, nt, :, 0:D],
                        ps[:].rearrange("p (h d) -> p h d", h=H),
                        mfT[:, nt:nt + 1], None, op0=OP.mult)
                    nc.vector.tensor_scalar(
                        Vt[:, nt, :, D], ones8[:], mfT[:, nt:nt + 1], None,
                        op0=OP.mult)

            # ---------- main-loop pools (reuse setup space) ----------
            pMAIN = ctx.enter_context(tc.tile_pool(name="pMAIN", bufs=1))
            pPT = ctx.enter_context(tc.tile_pool(name="pPT", bufs=1))
            pOUT = ctx.enter_context(tc.tile_pool(name="pOUT", bufs=2))
            pPP = ctx.enter_context(tc.tile_pool(name="pPP", bufs=3))
            At = [pMAIN.tile([128, nfull], BF16, tag="a", bufs=QT, name=f"a{i}")
                  for i in range(QT)]
            ONt = [pMAIN.tile([128, C], BF16, tag="on", bufs=QT, name=f"on{i}")
                   for i in range(QT)]
            OTt = [pMAIN.tile([128, nq], BF16, tag="ot", bufs=CT, name=f"ot{i}")
                   for i in range(CT)]
            RZt = [pMAIN.tile([128, H], FP32, tag="rz", bufs=QT, name=f"rz{i}")
                   for i in range(QT)]
            LZt = [pMAIN.tile([128, H], FP32, tag="lz", bufs=QT, name=f"lz{i}")
                   for i in range(QT)]

            # ---------- main loop over head pairs ----------
            for hp in range(HP):
                for hh in range(2):
                    h = 2 * hp + hh
                    b0 = hh * 64
                    # S^T -> exp -> P~^T  [k-tile, q]
                    PTt = pPT.tile([128, NT, nq], BF16, tag="pt")
                    for kt in range(NT):
                        ps = sp.tile([128, nq], FP32, tag="s")
                        for q5 in range(NQ5):
                            nc.tensor.matmul(
                                ps[:, q5 * W5:(q5 + 1) * W5],
                                KTt[hp][b0:b0 + 64, kt * 128:(kt + 1) * 128],
                                QTt[hp][b0:b0 + 64, q5 * W5:(q5 + 1) * W5])
                        nc.scalar.activation(PTt[:, kt, :], ps[:], AF.Exp)
                    # [V|m]^T @ P~^T -> O_unnorm^T [d+1, q] and Z row
                    otmp = pPP.tile([D + 1, nq], FP32, tag="ox")
                    for q5 in range(NQ5):
                        ov = ovp.tile([D + 1, W5], FP32, tag="ov")
                        for kt in range(NT):
                            nc.tensor.matmul(
                                ov[:],
                                Vt[:, kt, h, :],
                                PTt[:, kt, q5 * W5:(q5 + 1) * W5],
                                start=(kt == 0), stop=(kt == NT - 1))
                        nc.vector.tensor_copy(
                            otmp[:, q5 * W5:(q5 + 1) * W5], ov[:])
                    # transpose per q-tile -> [128, d+1]; rz; lz; O-normalize
                    for qt in range(QT):
                        tpo = pvp.tile([128, D + 1], FP32, tag="pv")
                        nc.tensor.transpose(
                            tpo[:], otmp[:, qt * 128:(qt + 1) * 128],
                            id32[0:D + 1, 0:D + 1])
                        nc.vector.reciprocal(RZt[qt][:, h:h + 1],
                                             tpo[:, D:D + 1])
                        nc.scalar.activation(LZt[qt][:, h:h + 1],
                                             RZt[qt][:, h:h + 1], AF.Ln)
                        nc.vector.tensor_scalar(
                            ONt[qt][:, h * D:(h + 1) * D], tpo[:, 0:D],
                            RZt[qt][:, h:h + 1], None, op0=OP.mult)
                # [q,k] side for the pair: S -> exp -> A += P~ * rz
                for qt in range(QT):
                    for kc in range(KC):
                        tl = [sp.tile([128, KCW], FP32, tag="s", name=f"t{i}")
                              for i in range(2)]
                        for hh in range(2):
                            b0 = hh * 64
                            for c5 in range(NS5):
                                o = kc * KCW + c5 * S5
                                nc.tensor.matmul(
                                    tl[hh][:, c5 * S5:(c5 + 1) * S5],
                                    QTt[hp][b0:b0 + 64,
                                            qt * 128:(qt + 1) * 128],
                                    KTt[hp][b0:b0 + 64, o:o + S5])
                        for hh in range(2):
                            h = 2 * hp + hh
                            asl = At[qt][:, kc * KCW:(kc + 1) * KCW]
                            if h == 0:
                                nc.scalar.activation(asl, tl[hh][:], AF.Exp,
                                                     bias=LZt[qt][:, h:h + 1])
                            else:
                                pp = pPP.tile([128, KCW], BF16, tag="pp")
                                nc.scalar.activation(pp[:], tl[hh][:], AF.Exp,
                                                     bias=LZt[qt][:, h:h + 1])
                                nc.vector.tensor_tensor(asl, pp[:], asl,
                                                        op=OP.add)

            # ---------- tail: mask A + DMA; proj ----------
            for qt in range(QT):
                nc.vector.tensor_tensor(At[qt][:], At[qt][:], maskbc[:],
                                        op=OP.mult)
                nc.sync.dma_start(out_a[qt * 128:(qt + 1) * 128, :], At[qt][:])

            for qt in range(QT):
                for ct in range(CT):
                    tp = pvp.tile([128, 128], BF16, tag="pv")
                    nc.tensor.transpose(
                        tp[:], ONt[qt][:, ct * 128:(ct + 1) * 128], id16[:])
                    nc.vector.tensor_copy(
                        OTt[ct][:, qt * 128:(qt + 1) * 128], tp[:])
            for qt in range(QT):
                pj = pvp.tile([128, C], FP32, tag="pv")
                for ct in range(CT):
                    nc.tensor.matmul(pj[:], OTt[ct][:, qt * 128:(qt + 1) * 128],
                                     WPt[ct][:], start=(ct == 0),
                                     stop=(ct == CT - 1))
                ob = pOUT.tile([128, C], FP32, tag="ob")
                nc.vector.tensor_tensor(ob[:], pj[:], bpb[:], op=OP.add)
                nc.sync.dma_start(out_o[qt * 128:(qt + 1) * 128, :], ob[:])

    nc.compile()
    return nc


def _get_program(nfull=NFULL, nq=NQ):
    key = (nfull, nq)
    if key not in _prog_cache:
        _prog_cache[key] = _build_program(nfull, nq)
    return _prog_cache[key]


def _run(x, mask, W_qkv, b_qkv, W_proj, b_proj, nfull, nq):
    from concourse.bass_utils import run_bass_kernel_spmd

    nbatch = x.shape[0]
    halves = N_CORES // nbatch
    nc = _get_program(nfull, nq)
    in_maps = []
    for c in range(N_CORES):
        b, j = c // halves, c % halves
        in_maps.append({
            "xb": np.roll(x[b], -j * nq, axis=0) if j else x[b],
            "maskv": np.roll(mask[b], -j * nq) if j else mask[b],
            "wqkv": W_qkv, "bqkv": b_qkv, "wproj": W_proj, "bproj": b_proj,
        })
    res = run_bass_kernel_spmd(nc, in_maps, list(range(N_CORES)))

    out = np.empty((nbatch, nfull, C), np.float32)
    attn = np.empty((nbatch, nfull, nfull), np.float32)
    for c in range(N_CORES):
        b, j = c // halves, c % halves
        out[b, j * nq:(j + 1) * nq] = res.results[c]["out_o"]
        a = res.results[c]["out_a"].astype(np.float32) / H
        if j:
            a = np.roll(a, j * nq, axis=1)
        attn[b, j * nq:(j + 1) * nq] = a
    return out, attn


def kernel(x, mask, W_qkv, b_qkv, W_proj, b_proj):
    return _run(np.asarray(x, np.float32), np.asarray(mask, np.int32),
                np.asarray(W_qkv, np.float32), np.asarray(b_qkv, np.float32),
                np.asarray(W_proj, np.float32), np.asarray(b_proj, np.float32),
                NFULL, NQ)


# revision 13
# speedup vs baseline: 3.1208x; 3.1208x over previous
"""Trainium2 Bass kernel for nn_MultiHeadAttention (B=4, N=2048, C=512, H=8).

Returns (out [B,N,C] f32, attn_mean [B,N,N] f32) like the reference.

Sharding: 8 cores = 4 batches x 2 query-halves. Each core computes all 8
heads for its (batch, 1024-query-row) slice; no collectives. The query-half
selection is done by rolling x/mask on the host so the SPMD program is
identical on every core (each core handles rows [0:NQ) of its rolled input;
attn_mean columns are rolled back on the host).

Per-core dataflow (matmuls bf16, PSUM accumulation f32):
  setup: x^T via PE transpose; Q^T (x0.125, +bias), K^T (+bias) in [d, n]
         layout; V masked by the key mask with an extra mask column (so the
         P@V matmul also yields the masked softmax denominator Z).
  per head:  S^T[k,q] matmul -> exp -> P~^T (unnormalized, bf16)
             P~^T @ [V|m] -> O_unnorm and Z; rz = 1/Z; O = O_unnorm * rz
             S[q,k] matmul -> exp -> P~; A[q,k] += P~ * rz   (fused DVE op)
  tail:  A *= mask (zero masked columns), DMA out (bf16; host /8 -> mean)
         out = (O @ W_proj + b_proj) via PE transposes of O.
"""

import numpy as np
import ml_dtypes

B, NFULL, C = 4, 2048, 512
H, D = 8, 64
NQ = NFULL // 2  # query rows per core
N_CORES = 8
SCALE = D ** -0.5

_prog_cache = {}


def _pin_act_tables():
    import concourse.bacc as bacc_mod
    import concourse.mybir as mybir
    if getattr(bacc_mod, "_act_tables_pinned", False):
        return
    orig = bacc_mod.get_activation_tables

    def pinned(arch):
        t = dict(orig(arch))
        keep = "natural_log_exp_and_others"
        drop = {mybir.ActivationFunctionType.Exp,
                mybir.ActivationFunctionType.Ln}
        if keep in t:
            for name in t:
                if name != keep:
                    t[name] = t[name] - drop
        return t

    bacc_mod.get_activation_tables = pinned
    bacc_mod._act_tables_pinned = True


def _build_program(nfull=NFULL, nq=NQ):
    import concourse.bass as bass
    import concourse.tile as tile
    import concourse.mybir as mybir
    from concourse import bacc
    from concourse.masks import make_identity
    _pin_act_tables()

    dt = mybir.dt
    FP32 = dt.float32
    BF16 = dt.bfloat16
    AF = mybir.ActivationFunctionType
    OP = mybir.AluOpType

    NT = nfull // 128      # k/v row tiles
    CT = C // 128          # channel tiles (4)
    QT = nq // 128         # query tiles per core
    HP = H // 2            # head pairs (4)
    KCW = min(1024, nfull)       # k chunk width for the [q,k] side
    KC = nfull // KCW            # chunks
    W5 = min(512, nq)            # matmul free width for [d,n] outputs
    NQ5 = nq // W5
    S5 = min(512, KCW)           # matmul free width for S[q,k]
    NS5 = KCW // S5

    nc = bacc.Bacc("TRN2", target_bir_lowering=False, debug=False,
                   num_devices=N_CORES)

    xb = nc.dram_tensor("xb", [nfull, C], FP32, kind="ExternalInput").ap()
    maskv = nc.dram_tensor("maskv", [nfull], dt.int32, kind="ExternalInput").ap()
    wqkv = nc.dram_tensor("wqkv", [C, 3 * C], FP32, kind="ExternalInput").ap()
    bqkv = nc.dram_tensor("bqkv", [3 * C], FP32, kind="ExternalInput").ap()
    wproj = nc.dram_tensor("wproj", [C, C], FP32, kind="ExternalInput").ap()
    bproj = nc.dram_tensor("bproj", [C], FP32, kind="ExternalInput").ap()
    out_o = nc.dram_tensor("out_o", [nq, C], FP32, kind="ExternalOutput").ap()
    out_a = nc.dram_tensor("out_a", [nq, nfull], BF16, kind="ExternalOutput").ap()

    with tile.TileContext(nc) as tc:
        from contextlib import ExitStack
        ctx = ExitStack()
        with ctx:
            # ---------- persistent pools (each tag: its own bufs) ----------
            pers = ctx.enter_context(tc.tile_pool(name="pers", bufs=1))

            # PSUM pools
            sp = ctx.enter_context(tc.tile_pool(name="sp", bufs=2, space="PSUM"))
            pvp = ctx.enter_context(tc.tile_pool(name="pvp", bufs=2, space="PSUM"))
            ovp = ctx.enter_context(tc.tile_pool(name="ovp", bufs=2, space="PSUM"))

            id32 = pers.tile([128, 128], FP32, tag="id32")
            make_identity(nc, id32[:])
            id16 = pers.tile([128, 128], BF16, tag="id16")
            make_identity(nc, id16[:])

            mfT = pers.tile([128, NT], FP32, tag="mfT")     # mask [k%128, kt]
            maskbc = pers.tile([128, nfull], BF16, tag="maskbc")
            ones8 = pers.tile([128, H], BF16, tag="ones8")
            bpb = pers.tile([128, C], FP32, tag="bpb")      # bproj broadcast
            onesrow = pers.tile([1, nfull], BF16, tag="onesrow")
            nc.vector.memset(ones8[:], 1.0)
            nc.vector.memset(onesrow[:], 1.0)

            QTt = [pers.tile([128, nq], BF16, tag="qt", bufs=HP, name=f"qt{i}")
                   for i in range(HP)]
            KTt = [pers.tile([128, nfull], BF16, tag="kt", bufs=HP, name=f"kt{i}")
                   for i in range(HP)]
            Vt = pers.tile([128, NT, H, D + 1], BF16, tag="vt")
            WPt = [pers.tile([128, C], BF16, tag="wp", bufs=CT, name=f"wp{i}")
                   for i in range(CT)]

            # ---------- setup (transient pools) ----------
            with tc.tile_pool(name="stage", bufs=2) as stg, \
                 tc.tile_pool(name="sxt", bufs=1) as sxt, \
                 tc.tile_pool(name="svec", bufs=1) as svec:

                # mask: strided [128, NT] + row + broadcast
                mi = svec.tile([128, NT], dt.int32, tag="mi")
                nc.sync.dma_start(mi[:], maskv.rearrange("(t p) -> p t", p=128))
                nc.vector.tensor_copy(mfT[:], mi[:])
                mri = svec.tile([1, nfull], dt.int32, tag="mri")
                nc.sync.dma_start(mri[:], maskv.unsqueeze(0))
                mrf = svec.tile([1, nfull], FP32, tag="mrf")
                nc.vector.tensor_copy(mrf[:], mri[:])
                mrow16 = svec.tile([1, nfull], BF16, tag="mrow16")
                nc.vector.tensor_copy(mrow16[:], mrf[:])
                # broadcast along partitions via ones-column x row matmuls
                bw = min(512, nfull)
                for kb in range(nfull // bw):
                    bb = pvp.tile([128, bw], FP32, tag="pv", name="bb")
                    nc.tensor.matmul(bb[:], onesrow[0:1, 0:128],
                                     mrow16[0:1, kb * bw:(kb + 1) * bw])
                    nc.vector.tensor_copy(maskbc[:, kb * bw:(kb + 1) * bw],
                                          bb[:])
                bprow = svec.tile([1, C], FP32, tag="bprow")
                nc.sync.dma_start(bprow[:], bproj.unsqueeze(0))
                ones32 = svec.tile([1, 128], FP32, tag="ones32")
                nc.vector.memset(ones32[:], 1.0)
                bb2 = pvp.tile([128, C], FP32, tag="pv", name="bb2")
                nc.tensor.matmul(bb2[:], ones32[:], bprow[:])
                nc.vector.tensor_copy(bpb[:], bb2[:])

                # qkv bias vectors [128, 12] (column t = b[128t:128(t+1)])
                bq12 = svec.tile([128, 3 * CT], FP32, tag="bq12")
                nc.sync.dma_start(bq12[:], bqkv.rearrange("(t p) -> p t", p=128))
                bqs = svec.tile([128, CT], FP32, tag="bqs")  # 0.125 * b_q
                nc.vector.tensor_scalar(bqs[:], bq12[:, 0:CT], SCALE, None,
                                        op0=OP.mult)

                # x -> SBUF, transpose -> xT (bf16)
                xT = [sxt.tile([128, nfull], BF16, tag="xT", bufs=CT, name=f"xT{i}")
                      for i in range(CT)]
                for nt in range(NT):
                    xs = stg.tile([128, C], FP32, tag="xs")
                    nc.sync.dma_start(xs[:], xb[nt * 128:(nt + 1) * 128, :])
                    for ct in range(CT):
                        tp = pvp.tile([128, 128], FP32, tag="pv")
                        nc.tensor.transpose(tp[:],
                                            xs[:, ct * 128:(ct + 1) * 128],
                                            id32[:])
                        nc.vector.tensor_copy(
                            xT[ct][:, nt * 128:(nt + 1) * 128], tp[:])

                # W_qkv -> bf16 (Q columns pre-scaled by 0.125)
                Wc = [sxt.tile([128, 3 * C], BF16, tag="wc", bufs=CT, name=f"wc{i}")
                      for i in range(CT)]
                bvrow = sxt.tile([1, C], BF16, tag="bvrow")
                bvf = svec.tile([1, 3 * C], FP32, tag="bvf")
                nc.sync.dma_start(bvf[:], bqkv.unsqueeze(0))
                nc.vector.tensor_copy(bvrow[:], bvf[0:1, 2 * C:3 * C])
                for ct in range(CT):
                    ws = stg.tile([128, 3 * C], FP32, tag="ws")
                    nc.sync.dma_start(ws[:], wqkv[ct * 128:(ct + 1) * 128, :])
                    nc.vector.tensor_scalar(Wc[ct][:, 0:C], ws[:, 0:C], SCALE,
                                            None, op0=OP.mult)
                    nc.vector.tensor_copy(Wc[ct][:, C:3 * C], ws[:, C:3 * C])

                # W_proj -> bf16
                for ct in range(CT):
                    wps = stg.tile([128, C], FP32, tag="wps")
                    nc.sync.dma_start(wps[:], wproj[ct * 128:(ct + 1) * 128, :])
                    nc.vector.tensor_copy(WPt[ct][:], wps[:])

                # Q^T tiles [d-pair 128, nq]: 0.125*(Wq^T x^T) + 0.125*bq
                # (0.125 pre-folded into Wc's Q columns and bqs)
                for j in range(HP):
                    ps = sp.tile([128, nq], FP32, tag="s")
                    for ct in range(CT):
                        for q5 in range(NQ5):
                            nc.tensor.matmul(
                                ps[:, q5 * W5:(q5 + 1) * W5],
                                Wc[ct][:, j * 128:(j + 1) * 128],
                                xT[ct][:, q5 * W5:(q5 + 1) * W5],
                                start=(ct == 0), stop=(ct == CT - 1))
                    nc.vector.tensor_scalar(QTt[j][:], ps[:],
                                            bqs[:, j:j + 1], None, op0=OP.add)

                # K^T tiles [d-pair 128, nfull]
                for j in range(HP):
                    for kc in range(nfull // nq):
                        ps = sp.tile([128, nq], FP32, tag="s")
                        for ct in range(CT):
                            for q5 in range(NQ5):
                                o = kc * nq + q5 * W5
                                nc.tensor.matmul(
                                    ps[:, q5 * W5:(q5 + 1) * W5],
                                    Wc[ct][:, C + j * 128:C + (j + 1) * 128],
                                    xT[ct][:, o:o + W5],
                                    start=(ct == 0), stop=(ct == CT - 1))
                        nc.vector.tensor_scalar(
                            KTt[j][:, kc * nq:(kc + 1) * nq], ps[:],
                            bq12[:, CT + j:CT + j + 1], None, op0=OP.add)

                # V tiles: masked, with mask column at d=64
                for nt in range(NT):
                    ps = pvp.tile([128, C], FP32, tag="pv")
                    for ct in range(CT):
                        nc.tensor.matmul(ps[:],
                                         xT[ct][:, nt * 128:(nt + 1) * 128],
                                         Wc[ct][:, 2 * C:3 * C],
                                         start=(ct == 0), stop=False)
                    nc.tensor.matmul(ps[:],
                                     onesrow[0:1, nt * 128:(nt + 1) * 128],
                                     bvrow[0:1, :], start=False, stop=True)
                    nc.vector.tensor_scalar(
                        Vt[:, nt, :, 0:D],
                        ps[:].rearrange("p (h d) -> p h d", h=H),
                        mfT[:, nt:nt + 1], None, op0=OP.mult)
                    nc.vector.tensor_scalar(
                        Vt[:, nt, :, D], ones8[:], mfT[:, nt:nt + 1], None,
                        op0=OP.mult)

            # ---------- main-loop pools (reuse setup space) ----------
            pMAIN = ctx.enter_context(tc.tile_pool(name="pMAIN", bufs=1))
            pPT = ctx.enter_context(tc.tile_pool(name="pPT", bufs=1))
            pOUT = ctx.enter_context(tc.tile_pool(name="pOUT", bufs=2))
            pPP = ctx.enter_context(tc.tile_pool(name="pPP", bufs=3))
            At = [pMAIN.tile([128, nfull], BF16, tag="a", bufs=QT, name=f"a{i}")
                  for i in range(QT)]
            ONt = [pMAIN.tile([128, C], BF16, tag="on", bufs=QT, name=f"on{i}")
                   for i in range(QT)]
            OTt = [pMAIN.tile([128, nq], BF16, tag="ot", bufs=CT, name=f"ot{i}")
                   for i in range(CT)]
            RZt = [pMAIN.tile([128, H], FP32, tag="rz", bufs=QT, name=f"rz{i}")
                   for i in range(QT)]
            LZt = [pMAIN.tile([128, H], FP32, tag="lz", bufs=QT, name=f"lz{i}")
                   for i in range(QT)]

            # ---------- main loop over head pairs ----------
            for hp in range(HP):
                for hh in range(2):
                    h = 2 * hp + hh
                    b0 = hh * 64
                    # S^T -> exp -> P~^T  [k-tile, q]
                    PTt = pPT.tile([128, NT, nq], BF16, tag="pt")
                    for kt in range(NT):
                        ps = sp.tile([128, nq], FP32, tag="s")
                        for q5 in range(NQ5):
                            nc.tensor.matmul(
                                ps[:, q5 * W5:(q5 + 1) * W5],
                                KTt[hp][b0:b0 + 64, kt * 128:(kt + 1) * 128],
                                QTt[hp][b0:b0 + 64, q5 * W5:(q5 + 1) * W5])
                        nc.scalar.activation(PTt[:, kt, :], ps[:], AF.Exp)
                    # [V|m]^T @ P~^T -> O_unnorm^T [d+1, q] and Z row
                    otmp = pPP.tile([D + 1, nq], FP32, tag="ox")
                    for q5 in range(NQ5):
                        ov = ovp.tile([D + 1, W5], FP32, tag="ov")
                        for kt in range(NT):
                            nc.tensor.matmul(
                                ov[:],
                                Vt[:, kt, h, :],
                                PTt[:, kt, q5 * W5:(q5 + 1) * W5],
                                start=(kt == 0), stop=(kt == NT - 1))
                        nc.vector.tensor_copy(
                            otmp[:, q5 * W5:(q5 + 1) * W5], ov[:])
                    # transpose per q-tile -> [128, d+1]; rz; lz; O-normalize
                    for qt in range(QT):
                        tpo = pvp.tile([128, D + 1], FP32, tag="pv")
                        nc.tensor.transpose(
                            tpo[:], otmp[:, qt * 128:(qt + 1) * 128],
                            id32[0:D + 1, 0:D + 1])
                        nc.vector.reciprocal(RZt[qt][:, h:h + 1],
                                             tpo[:, D:D + 1])
                        nc.scalar.activation(LZt[qt][:, h:h + 1],
                                             RZt[qt][:, h:h + 1], AF.Ln)
                        nc.vector.tensor_scalar(
                            ONt[qt][:, h * D:(h + 1) * D], tpo[:, 0:D],
                            RZt[qt][:, h:h + 1], None, op0=OP.mult)
                # [q,k] side for the pair: S -> exp -> A += P~ * rz
                for qt in range(QT):
                    for kc in range(KC):
                        tl = [sp.tile([128, KCW], FP32, tag="s", name=f"t{i}")
                              for i in range(2)]
                        for hh in range(2):
                            b0 = hh * 64
                            for c5 in range(NS5):
                                o = kc * KCW + c5 * S5
                                nc.tensor.matmul(
                                    tl[hh][:, c5 * S5:(c5 + 1) * S5],
                                    QTt[hp][b0:b0 + 64,
                                            qt * 128:(qt + 1) * 128],
                                    KTt[hp][b0:b0 + 64, o:o + S5])
                        for hh in range(2):
                            h = 2 * hp + hh
                            asl = At[qt][:, kc * KCW:(kc + 1) * KCW]
                            if h == 0:
                                nc.scalar.activation(asl, tl[hh][:], AF.Exp,
                                                     bias=LZt[qt][:, h:h + 1])
                            else:
                                pp = pPP.tile([128, KCW], BF16, tag="pp")
                                nc.scalar.activation(pp[:], tl[hh][:], AF.Exp,
                                                     bias=LZt[qt][:, h:h + 1])
                                nc.vector.tensor_tensor(asl, pp[:], asl,
                                                        op=OP.add)

            # ---------- tail: mask A + DMA; proj ----------
            for qt in range(QT):
                nc.vector.tensor_tensor(At[qt][:], At[qt][:], maskbc[:],
                                        op=OP.mult)
                nc.sync.dma_start(out_a[qt * 128:(qt + 1) * 128, :], At[qt][:])

            for qt in range(QT):
                for ct in range(CT):
                    tp = pvp.tile([128, 128], BF16, tag="pv")
                    nc.tensor.transpose(
                        tp[:], ONt[qt][:, ct * 128:(ct + 1) * 128], id16[:])
                    nc.vector.tensor_copy(
                        OTt[ct][:, qt * 128:(qt + 1) * 128], tp[:])
            for qt in range(QT):
                pj = pvp.tile([128, C], FP32, tag="pv")
                for ct in range(CT):
                    nc.tensor.matmul(pj[:], OTt[ct][:, qt * 128:(qt + 1) * 128],
                                     WPt[ct][:], start=(ct == 0),
                                     stop=(ct == CT - 1))
                ob = pOUT.tile([128, C], FP32, tag="ob")
                nc.vector.tensor_tensor(ob[:], pj[:], bpb[:], op=OP.add)
                nc.sync.dma_start(out_o[qt * 128:(qt + 1) * 128, :], ob[:])

    nc.compile()
    return nc


def _get_program(nfull=NFULL, nq=NQ):
    key = (nfull, nq)
    if key not in _prog_cache:
        _prog_cache[key] = _build_program(nfull, nq)
    return _prog_cache[key]


def _run(x, mask, W_qkv, b_qkv, W_proj, b_proj, nfull, nq):
    from concourse.bass_utils import run_bass_kernel_spmd

    nbatch = x.shape[0]
    halves = N_CORES // nbatch
    nc = _get_program(nfull, nq)
    in_maps = []
    for c in range(N_CORES):
        b, j = c // halves, c % halves
        in_maps.append({
            "xb": np.roll(x[b], -j * nq, axis=0) if j else x[b],
            "maskv": np.roll(mask[b], -j * nq) if j else mask[b],
            "wqkv": W_qkv, "bqkv": b_qkv, "wproj": W_proj, "bproj": b_proj,
        })
    res = run_bass_kernel_spmd(nc, in_maps, list(range(N_CORES)))

    out = np.empty((nbatch, nfull, C), np.float32)
    attn = np.empty((nbatch, nfull, nfull), np.float32)
    for c in range(N_CORES):
        b, j = c // halves, c % halves
        out[b, j * nq:(j + 1) * nq] = res.results[c]["out_o"]
        a = res.results[c]["out_a"].astype(np.float32) / H
        if j:
            a = np.roll(a, j * nq, axis=1)
        attn[b, j * nq:(j + 1) * nq] = a
    return out, attn


def kernel(x, mask, W_qkv, b_qkv, W_proj, b_proj):
    return _run(np.asarray(x, np.float32), np.asarray(mask, np.int32),
                np.asarray(W_qkv, np.float32), np.asarray(b_qkv, np.float32),
                np.asarray(W_proj, np.float32), np.asarray(b_proj, np.float32),
                NFULL, NQ)
